# revision 7
# baseline (speedup 1.0000x reference)
"""Multi-layer bidirectional binary-tree LSTM on 8 Trainium2 NeuronCores. v2

Decomposition (validated against the jax reference by a numpy golden model):
  - Tree levels 11..17 sharded 8-way along the node axis (contiguous chunks
    keep parent/child ranges core-local); levels 0..10 replicated.
  - One AllGather per layer of the level-11 (h,c) states feeds the up-pass
    top; one ReduceScatter(max) per layer hands each core its level-10
    parent block for the down pass (all copies identical, max == identity).
  - All compute is feature-major (g.T = W @ x.T), gates packed into
    128-partition PSUM chunks, matmuls in float32r, child states stored
    parity-split so one matmul pass covers both children.
  - v2: layer-0 up sweep transposes x once and stores xT to DRAM (xt0);
    all other sweeps stream feature-major directly. Outputs leave the
    device feature-major (host transposes during assembly).
"""
import hashlib
import sys
sys.path.insert(0, '/opt/trn_rl_repo')
import numpy as np

_SRC_HASH = hashlib.md5(open(__file__, 'rb').read()).hexdigest()[:10]

D = 18
N = 2 ** D - 1
FEAT = 256
H = 64
SH_LV = 11
NCORES = 8
TOP = 2 ** SH_LV - 1
DEEP = sum(2 ** (l - 3) for l in range(SH_LV, D))
ROWS = TOP + DEEP
TB = 512                    # tile size (nodes) for every sweep

_cache = {}


def _deep_off(l):
    return TOP + sum(2 ** (j - 3) for j in range(SH_LV, l))


# ---------------------------------------------------------------- host prep
def _prep_up(p):
    Wx, bx = np.asarray(p['Wx'], np.float32), np.asarray(p['bx'], np.float32)
    Wl, bl = np.asarray(p['Wl'], np.float32), np.asarray(p['bl'], np.float32)
    Wr, br = np.asarray(p['Wr'], np.float32), np.asarray(p['br'], np.float32)
    Wpx, bpx = np.asarray(p['Wpx'], np.float32), np.asarray(p['bpx'], np.float32)
    bt = bx + bl + br
    # C0=[i,o] C1=[fl,fr] C2=[u,r] -- original gate order
    wx = np.ascontiguousarray(
        np.concatenate([Wx.T, Wpx.T], axis=1), np.float32)            # [in_d,448]
    wlr = np.ascontiguousarray(
        np.concatenate([Wl.T, Wr.T], axis=0), np.float32)              # [128,384]
    bias = np.zeros((128, 4), np.float32)
    bias[:, 0] = bt[0:128]
    bias[:, 1] = bt[128:256]
    bias[0:64, 2] = bt[256:320]
    bias[64:128, 2] = 0.5 * bt[320:384]
    bias[0:64, 3] = bpx
    return wx.astype(np.float16), wlr.astype(np.float16), bias


def _prep_dn(p):
    Wx, bx = np.asarray(p['Wx'], np.float32), np.asarray(p['bx'], np.float32)
    Wh, bh = np.asarray(p['Wh'], np.float32), np.asarray(p['bh'], np.float32)
    Wpx, bpx = np.asarray(p['Wpx'], np.float32), np.asarray(p['bpx'], np.float32)
    bt = bx + bh
    perm = np.r_[0:128, 256:320, 128:192, 192:256]   # C0=[i,o] C1=[r,f] C2=[u]
    wx = np.ascontiguousarray(
        np.concatenate([Wx[perm].T, Wpx.T], axis=1), np.float32)      # [in_d,384]
    wh = np.ascontiguousarray(
        np.concatenate([Wh[perm].T, np.zeros((64, 64), np.float32)], axis=1),
        np.float32)                                                    # [64,384]
    bias = np.zeros((128, 3), np.float32)
    bias[:, 0] = bt[0:128]
    bias[0:64, 1] = bt[256:320]
    bias[64:128, 1] = bt[128:192]
    bias[0:64, 2] = bt[192:256]
    bias[64:128, 2] = bpx
    return wx.astype(np.float16), wh.astype(np.float16), bias


def _slice_features(x_full):
    out = []
    for k in range(NCORES):
        parts = [x_full[0:TOP]]
        for l in range(SH_LV, D):
            m = 2 ** (l - 3)
            base = 2 ** l - 1
            parts.append(x_full[base + k * m: base + (k + 1) * m])
        out.append(np.ascontiguousarray(np.concatenate(parts, axis=0)))
    return out


def _assemble(results):
    """results: per-core outT [128, ROWS] (rows 0:64 up-h, 64:128 dn-h,
    cols in [top | deep] order) -> [N, 128] node-major."""
    results = [np.asarray(r, np.float32) for r in results]
    full = np.empty((N, 128), np.float32)
    full[0:TOP] = results[0][:, 0:TOP].T
    for l in range(SH_LV, D):
        m = 2 ** (l - 3)
        base = 2 ** l - 1
        off = _deep_off(l)
        for k in range(NCORES):
            full[base + k * m: base + (k + 1) * m] = \
                results[k][:, off: off + m].T
    return full


# ------------------------------------------------------------- device build
def _build():
    import concourse.bacc as bacc
    import concourse.mybir as mybir
    import concourse.tile as tile

    F32 = mybir.dt.float32
    F32R = mybir.dt.float32r
    F16 = mybir.dt.float16
    core_ids = list(range(NCORES))

    nc = bacc.Bacc("TRN2", target_bir_lowering=False, debug=False,
                   num_devices=NCORES)

    nc.dram_tensor(f"bt_{_SRC_HASH}", [1, 1], F32, kind="ExternalInput")
    feats = nc.dram_tensor("feats", [ROWS, FEAT], F32, kind="ExternalInput")
    ident = nc.dram_tensor("ident", [128, 128], F32, kind="ExternalInput")
    uscale = nc.dram_tensor("uscale", [128, 1], F32, kind="ExternalInput")
    wts = {}
    for L in (0, 1):
        in_d = FEAT if L == 0 else 128
        wts[f"u{L}_wx"] = nc.dram_tensor(f"u{L}_wx", [in_d, 448], F16,
                                         kind="ExternalInput")
        wts[f"u{L}_wlr"] = nc.dram_tensor(f"u{L}_wlr", [128, 384], F16,
                                          kind="ExternalInput")
        wts[f"u{L}_b"] = nc.dram_tensor(f"u{L}_b", [128, 4], F32,
                                        kind="ExternalInput")
        wts[f"d{L}_wx"] = nc.dram_tensor(f"d{L}_wx", [in_d, 384], F16,
                                         kind="ExternalInput")
        wts[f"d{L}_wh"] = nc.dram_tensor(f"d{L}_wh", [64, 384], F16,
                                         kind="ExternalInput")
        wts[f"d{L}_b"] = nc.dram_tensor(f"d{L}_b", [128, 3], F32,
                                        kind="ExternalInput")
    outT = nc.dram_tensor("outT", [128, ROWS], F16, kind="ExternalOutput")
    xt0 = nc.dram_tensor("xt0", [FEAT, ROWS], F16)
    xt1 = nc.dram_tensor("xt1", [128, ROWS], F16)
    ag_in, ag_out, rs_in, rs_out = {}, {}, {}, {}
    for L in (0, 1):
        ag_in[L] = nc.dram_tensor(f"agin{L}", [256, 128], F16)
        ag_out[L] = nc.dram_tensor(f"agout{L}", [2048, 128], F16,
                                   addr_space="Shared")
        rs_in[L] = nc.dram_tensor(f"rsin{L}", [1024, 128], F16)
        rs_out[L] = nc.dram_tensor(f"rsout{L}", [128, 128], F16)

    with tile.TileContext(nc) as tc:
        with tc.tile_pool(name="const", bufs=1) as cpool:
            id_sb = cpool.tile([128, 128], F32)
            nc.sync.dma_start(id_sb[:], ident[:])
            usc_sb = cpool.tile([128, 1], F32)
            nc.sync.dma_start(usc_sb[:], uscale[:])
            W = {"ident": id_sb, "uscale": usc_sb}
            for L in (0, 1):
                in_d = FEAT if L == 0 else 128
                nk = in_d // 128
                t = cpool.tile([128, nk * 448], F16, tag=f"u{L}_wx")
                nc.sync.dma_start(
                    t[:].rearrange("p (k m) -> p k m", k=nk),
                    wts[f"u{L}_wx"].ap().rearrange("(k p) m -> p k m", p=128))
                W[f"u{L}_wx"] = t
                t = cpool.tile([128, 384], F16, tag=f"u{L}_wlr")
                nc.sync.dma_start(t[:], wts[f"u{L}_wlr"][:])
                W[f"u{L}_wlr"] = t
                t = cpool.tile([128, 4], F32, tag=f"u{L}_b")
                nc.sync.dma_start(t[:], wts[f"u{L}_b"][:])
                W[f"u{L}_b"] = t
                t = cpool.tile([128, nk * 384], F16, tag=f"d{L}_wx")
                nc.sync.dma_start(
                    t[:].rearrange("p (k m) -> p k m", k=nk),
                    wts[f"d{L}_wx"].ap().rearrange("(k p) m -> p k m", p=128))
                W[f"d{L}_wx"] = t
                t = cpool.tile([64, 384], F16, tag=f"d{L}_wh")
                nc.sync.dma_start(t[:], wts[f"d{L}_wh"][:])
                W[f"d{L}_wh"] = t
                t = cpool.tile([128, 3], F32, tag=f"d{L}_b")
                nc.sync.dma_start(t[:], wts[f"d{L}_b"][:])
                W[f"d{L}_b"] = t

            for L in (0, 1):
                xin = xt0 if L == 0 else xt1
                xout = xt1 if L == 0 else outT
                _emit_up(nc, tc, mybir, L, W, xin, xt0, xout,
                         ag_in[L], ag_out[L], core_ids, feats=feats)
                _emit_dn(nc, tc, mybir, L, W, xin, xt0, xout,
                         rs_in[L], rs_out[L], core_ids)

    nc.compile()
    return nc


def _load_xt(nc, mybir, pools, L, xin, src0, n, ne):
    """Return xt sbuf tile [128, nk*ne] fp16, chunk-major, loaded from the
    feature-major fp16 DRAM image (xt0 for L0, xt1 for L1)."""
    F16 = mybir.dt.float16
    xpool = pools[0]
    nk = (FEAT if L == 0 else 128) // 128
    xt = xpool.tile([128, nk * ne], F16, tag="xt")
    if ne > n:
        nc.vector.memset(xt[:], 0.0)
    nc.sync.dma_start(
        xt[:].rearrange("p (k c) -> p k c", k=nk)[:, :, 0:n],
        xin.ap().rearrange("(k p) c -> p k c", p=128)[:, :, src0:src0 + n])
    return xt


def _emit_prepass(nc, mybir, W, feats, xt0, ppools, c0, c1, pidx=[0]):
    """Transpose node-major fp32 features[c0:c1] into the feature-major fp16
    xt0 image, 512 nodes per iteration (batched DMAs)."""
    F32 = mybir.dt.float32
    F16 = mybir.dt.float16
    ident = W["ident"]
    xpool, tpool = ppools
    for base in range(c0, c1, 512):
        nb = min(512, c1 - base)
        sc = (nb + 127) // 128
        xsb = xpool.tile([128, 4 * FEAT], F32, tag="pxs")
        if nb == 512:
            nc.sync.dma_start(
                xsb[:].rearrange("p (s f) -> p s f", s=4),
                feats[base:base + nb, :].rearrange("(s p) f -> p s f", p=128))
        else:
            for si in range(sc):
                sl = min(128, nb - si * 128)
                nc.sync.dma_start(
                    xsb[0:sl, si * FEAT:si * FEAT + FEAT],
                    feats[base + si * 128: base + si * 128 + sl, :])
        xtp = xpool.tile([128, 2 * 512], F16, tag="pxt")
        for k in range(2):
            tp = tpool.tile([128, 512], F32, tag="tp")
            for si in range(sc):
                sl = min(128, nb - si * 128)
                nc.tensor.transpose(
                    tp[:, si * 128: si * 128 + sl],
                    xsb[0:sl, si * FEAT + k * 128: si * FEAT + (k + 1) * 128],
                    ident[0:sl, 0:sl])
            if pidx[0] % 2 == 0:
                nc.vector.tensor_copy(xtp[:, k * 512:k * 512 + nb],
                                      tp[:, 0:nb])
            else:
                nc.scalar.copy(xtp[:, k * 512:k * 512 + nb], tp[:, 0:nb])
            pidx[0] += 1
        nc.sync.dma_start(
            xt0.ap().rearrange("(k p) c -> p k c", p=128)[:, :,
                                                          base:base + nb],
            xtp[:].rearrange("p (k c) -> p k c", k=2)[:, :, 0:nb])


def _up_tile(nc, mybir, pools, L, W, xin, xout,
             n, src0, child, st_dst, outcol0):
    """Up-pass tile of n nodes (ops padded to ne cols).
    child: None | (h_tile, c_tile, col0); st_dst: None | (h_t, c_t, pcol0)."""
    F32 = mybir.dt.float32
    F16 = mybir.dt.float16
    AF = mybir.ActivationFunctionType
    OP = mybir.AluOpType
    xpool, (gpool, gpool1), epool = pools
    in_d = FEAT if L == 0 else 128
    nk = in_d // 128
    ne = max(n, 8)
    wx = W[f"u{L}_wx"]
    wlr = W[f"u{L}_wlr"]
    bias = W[f"u{L}_b"]

    xt = _load_xt(nc, mybir, pools, L, xin, src0, n, ne)
    leaf = child is None
    if not leaf:
        ch_h = child[0][:, child[2]:child[2] + ne]
        ch_c = child[1][:, child[2]:child[2] + ne]

    g0 = gpool.tile([128, 512], F32, tag="gA")
    c0_ap = g0[:, 0:ne]
    if not leaf:
        g1 = gpool.tile([128, 512], F32, tag="gB")
        c1_ap = g1[:, 0:ne]
    g2 = gpool.tile([128, 512], F32, tag="gC")
    g3 = gpool1.tile([64, 512], F32, tag="gPx")
    c2_ap, px_ap = g2[:, 0:ne], g3[:, 0:ne]

    def mm(dst, m0, msz, with_child):
        for k in range(nk):
            nc.tensor.matmul(dst, wx[:, k * 448 + m0: k * 448 + m0 + msz],
                             xt[:, k * ne:(k + 1) * ne],
                             start=(k == 0),
                             stop=(k == nk - 1 and not with_child))
        if with_child:
            nc.tensor.matmul(dst, wlr[:, m0:m0 + msz], ch_h,
                             start=False, stop=True)

    mm(c0_ap, 0, 128, not leaf)                      # C0 = [i,o]
    if not leaf:
        mm(c1_ap, 128, 128, True)                    # C1 = [fl,fr]
    mm(c2_ap, 256, 128, not leaf)                    # C2 = [u,r]
    mm(px_ap, 384, 64, False)                        # px

    sio = epool.tile([128, TB], F16, tag="sio")
    nc.scalar.activation(sio[:, 0:ne], c0_ap, AF.Sigmoid, bias=bias[:, 0:1])
    if not leaf:
        sff = epool.tile([128, TB], F16, tag="sff")
        nc.scalar.activation(sff[:, 0:ne], c1_ap, AF.Sigmoid,
                             bias=bias[:, 1:2])
    tru = epool.tile([128, TB], F16, tag="tru")
    nc.scalar.activation(tru[:, 0:ne], c2_ap, AF.Tanh,
                         bias=bias[:, 2:3], scale=W["uscale"][:, 0:1])
    px = epool.tile([64, TB], F16, tag="px")
    nc.scalar.activation(px[:, 0:ne], px_ap, AF.Identity,
                         bias=bias[0:64, 3:4])

    # tru rows 0:64 = tanh(u), 64:128 = tanh(r/2). cf/tcc/dd on partitions
    # 64:128 so two-SBUF-input ops always have matching base partitions.
    cf = epool.tile([128, TB], F16, tag="cf")
    if leaf:
        nc.vector.tensor_mul(cf[64:128, 0:ne], sio[0:64, 0:ne],
                             tru[0:64, 0:ne])
    else:
        m1 = epool.tile([64, TB], F16, tag="m1")
        nc.vector.tensor_mul(m1[:, 0:ne], sio[0:64, 0:ne], tru[0:64, 0:ne])
        pr = epool.tile([128, TB], F16, tag="pr")
        nc.vector.tensor_mul(pr[:, 0:ne], sff[:, 0:ne], ch_c[:, :])
        nc.vector.tensor_add(cf[64:128, 0:ne], m1[:, 0:ne], pr[0:64, 0:ne])
        nc.vector.tensor_add(cf[64:128, 0:ne], cf[64:128, 0:ne],
                             pr[64:128, 0:ne])
    tcc = epool.tile([128, TB], F16, tag="tc")
    nc.scalar.activation(tcc[64:128, 0:ne], cf[64:128, 0:ne], AF.Tanh)
    hh = epool.tile([64, TB], F16, tag="hh")
    nc.vector.tensor_mul(hh[:, 0:ne], sio[64:128, 0:ne], tcc[64:128, 0:ne])
    dd = epool.tile([128, TB], F16, tag="dd")
    nc.vector.tensor_sub(dd[64:128, 0:ne], hh[:, 0:ne], px[:, 0:ne])
    ee = epool.tile([64, TB], F16, tag="ee")
    nc.vector.scalar_tensor_tensor(ee[:, 0:ne], tru[64:128, 0:ne], 1.0,
                                   dd[64:128, 0:ne], OP.add, OP.mult)
    of = epool.tile([64, TB], F16, tag="of")
    nc.vector.scalar_tensor_tensor(of[:, 0:ne], ee[:, 0:ne], 0.5,
                                   px[:, 0:ne], OP.mult, OP.add)
    if st_dst is not None:
        h_t, c_t, pc0 = st_dst
        hn = n // 2
        nc.vector.tensor_copy(h_t[0:64, pc0:pc0 + hn],
                              of[:, 0:n].rearrange("p (j s) -> p s j", s=2)[:, 0])
        nc.vector.tensor_copy(h_t[64:128, pc0:pc0 + hn],
                              of[:, 0:n].rearrange("p (j s) -> p s j", s=2)[:, 1])
        nc.vector.tensor_copy(c_t[0:64, pc0:pc0 + hn],
                              cf[64:128, 0:n].rearrange(
                                  "p (j s) -> p s j", s=2)[:, 0])
        nc.vector.tensor_copy(c_t[64:128, pc0:pc0 + hn],
                              cf[64:128, 0:n].rearrange(
                                  "p (j s) -> p s j", s=2)[:, 1])
    nc.sync.dma_start(xout[0:64, outcol0:outcol0 + n], of[:, 0:n])


def _dn_tile(nc, mybir, pools, L, W, xin, xout,
             n, src0, parent, st_dst, outcol0):
    """Down-pass tile. parent: None | (h_tile[64,w], c_tile[128,w], col0).
    st_dst: None | (h_tile, c_tile, c0)."""
    F32 = mybir.dt.float32
    F16 = mybir.dt.float16
    AF = mybir.ActivationFunctionType
    OP = mybir.AluOpType
    xpool, gpool, epool = pools
    in_d = FEAT if L == 0 else 128
    nk = in_d // 128
    ne = max(n, 8)
    wx = W[f"d{L}_wx"]
    wh = W[f"d{L}_wh"]
    bias = W[f"d{L}_b"]

    xt = _load_xt(nc, mybir, pools, L, xin, src0, n, ne)

    root = parent is None
    if not root:
        hp = ne // 2
        par_h = parent[0][:, parent[2]:parent[2] + hp]
        par_c = parent[1][:, parent[2]:parent[2] + hp]
        ph_b = par_h.to_broadcast((64, hp, 2))            # repeat-2 columns
        pc_b = par_c[64:128, :].to_broadcast((64, hp, 2))

    g0 = gpool.tile([128, 512], F32, tag="gA")
    g1 = gpool.tile([128, 512], F32, tag="gB")
    g2 = gpool.tile([128, 512], F32, tag="gC")
    c0_ap, c1_ap, c2_ap = g0[:, 0:ne], g1[:, 0:ne], g2[:, 0:ne]

    def mm(dst, m0, msz):
        for k in range(nk):
            nc.tensor.matmul(dst, wx[:, k * 384 + m0: k * 384 + m0 + msz],
                             xt[:, k * ne:(k + 1) * ne],
                             start=(k == 0), stop=(k == nk - 1 and root))
        if not root:
            nc.tensor.matmul(dst, wh[:, m0:m0 + msz], ph_b,
                             start=False, stop=True)

    mm(c0_ap, 0, 128)       # C0=[i,o]
    mm(c1_ap, 128, 128)     # C1=[r,f]
    mm(c2_ap, 256, 128)     # C2=[u,px]

    sio = epool.tile([128, TB], F16, tag="sio")
    nc.scalar.activation(sio[:, 0:ne], c0_ap, AF.Sigmoid, bias=bias[:, 0:1])
    sfr = epool.tile([128, TB], F16, tag="sff")
    nc.scalar.activation(sfr[:, 0:ne], c1_ap, AF.Sigmoid, bias=bias[:, 1:2])
    tu = epool.tile([64, TB], F16, tag="tru")
    nc.scalar.activation(tu[:, 0:ne], c2_ap[0:64, :], AF.Tanh,
                         bias=bias[0:64, 2:3])
    px = epool.tile([64, TB], F16, tag="px")
    nc.scalar.activation(px[:, 0:ne], c2_ap[64:128, :], AF.Identity,
                         bias=bias[64:128, 2:3])

    m1 = epool.tile([64, TB], F16, tag="m1")
    nc.vector.tensor_mul(m1[:, 0:ne], sio[0:64, 0:ne], tu[:, 0:ne])
    direct = st_dst is not None and n >= 8
    if direct:
        sh_t, sc_t, sc0 = st_dst
        cf_ap = sc_t[64:128, sc0:sc0 + ne]
    else:
        cft = epool.tile([128, TB], F16, tag="cf")
        cf_ap = cft[64:128, 0:ne]
    if root:
        nc.vector.tensor_copy(cf_ap, m1[:, 0:ne])
    else:
        tmp = epool.tile([64, TB], F16, tag="tmp")
        nc.vector.tensor_mul(
            tmp[:, 0:ne].rearrange("p (a b) -> p a b", b=2),
            sfr[64:128, 0:ne].rearrange("p (a b) -> p a b", b=2), pc_b)
        nc.vector.tensor_add(cf_ap, m1[:, 0:ne], tmp[:, 0:ne])
    tcc = epool.tile([128, TB], F16, tag="tc")
    nc.scalar.activation(tcc[64:128, 0:ne], cf_ap, AF.Tanh)
    hh = epool.tile([64, TB], F16, tag="hh")
    nc.vector.tensor_mul(hh[:, 0:ne], sio[64:128, 0:ne], tcc[64:128, 0:ne])
    dd = epool.tile([64, TB], F16, tag="dd")
    nc.vector.tensor_sub(dd[:, 0:ne], hh[:, 0:ne], px[:, 0:ne])
    ee = epool.tile([64, TB], F16, tag="ee")
    nc.vector.tensor_mul(ee[:, 0:ne], sfr[0:64, 0:ne], dd[:, 0:ne])
    if direct:
        of_ap = sh_t[:, sc0:sc0 + ne]
    else:
        oft = epool.tile([64, TB], F16, tag="of")
        of_ap = oft[:, 0:ne]
    nc.vector.tensor_add(of_ap, ee[:, 0:ne], px[:, 0:ne])
    if st_dst is not None and not direct:
        sh_t, sc_t, sc0 = st_dst
        nc.vector.tensor_copy(sh_t[:, sc0:sc0 + n], of_ap[:, 0:n])
        nc.vector.tensor_copy(sc_t[64:128, sc0:sc0 + n], cf_ap[:, 0:n])
    nc.sync.dma_start(xout[64:128, outcol0:outcol0 + n], of_ap[:, 0:n])


def _emit_up(nc, tc, mybir, L, W, xin, xt0, xout, agin, agout, core_ids,
             feats=None):
    F16 = mybir.dt.float16
    OP = mybir.AluOpType
    with (
        tc.tile_pool(name=f"upx{L}", bufs=3) as xpool,
        tc.tile_pool(name=f"upg{L}", bufs=2, space="PSUM") as gpool,
        tc.tile_pool(name=f"upg1{L}", bufs=1, space="PSUM") as gpool1,
        tc.tile_pool(name=f"uppp{L}", bufs=1, space="PSUM") as pppool,
        tc.tile_pool(name=f"upe{L}", bufs=2) as epool,
        tc.tile_pool(name=f"upst{L}", bufs=1) as stpool,
        tc.tile_pool(name=f"upp{L}", bufs=3) as ppx,
    ):
        pools = (xpool, (gpool, gpool1), epool)
        ppools = (ppx, pppool)
        st = {}
        for l in range(D - 1, SH_LV - 1, -1):           # deep, sharded
            m = 2 ** (l - 3)
            if L == 0:
                _emit_prepass(nc, mybir, W, feats, xt0, ppools,
                              _deep_off(l), _deep_off(l) + m)
            w = 8192 if l % 2 else 4096
            h_t = stpool.tile([128, w], F16, tag=f"uh{l % 2}")
            c_t = stpool.tile([128, w], F16, tag=f"uc{l % 2}")
            for t in range(0, m, TB):
                n = min(TB, m - t)
                child = None if l == D - 1 else (st[l + 1][0], st[l + 1][1], t)
                _up_tile(nc, mybir, pools, L, W, xin, xout,
                         n, _deep_off(l) + t, child,
                         (h_t, c_t, t // 2), _deep_off(l) + t)
            st[l] = (h_t, c_t)

        hb, cb = st[SH_LV]
        nc.sync.dma_start(agin[0:128, :], hb[:, 0:128])
        nc.sync.dma_start(agin[128:256, :], cb[:, 0:128])
        nc.gpsimd.collective_compute(
            "AllGather", OP.bypass, replica_groups=[core_ids],
            ins=[agin[:]], outs=[agout[:]])
        if L == 0:
            _emit_prepass(nc, mybir, W, feats, xt0, ppools, 0, TOP)

        l = SH_LV - 1                                    # gather-fed level 10
        h_t = stpool.tile([128, 4096], F16, tag=f"uh{l % 2}")
        c_t = stpool.tile([128, 4096], F16, tag=f"uc{l % 2}")
        for k in range(8):
            gh = xpool.tile([128, 128], F16, tag="gh")
            gc = xpool.tile([128, 128], F16, tag="gc")
            nc.sync.dma_start(gh[:], agout[k * 256: k * 256 + 128, :])
            nc.sync.dma_start(gc[:], agout[k * 256 + 128: k * 256 + 256, :])
            base = 2 ** l - 1
            _up_tile(nc, mybir, pools, L, W, xin, xout,
                     128, base + k * 128, (gh, gc, 0),
                     (h_t, c_t, k * 64), base + k * 128)
        st[l] = (h_t, c_t)

        for l in range(SH_LV - 2, -1, -1):               # replicated top
            nlev = 2 ** l
            base = 2 ** l - 1
            if l > 0:
                w = 8192 if l % 2 else 4096
                h_t = stpool.tile([128, w], F16, tag=f"uh{l % 2}")
                c_t = stpool.tile([128, w], F16, tag=f"uc{l % 2}")
                if nlev // 2 < 8:
                    nc.vector.memset(h_t[:, 0:8], 0.0)
                    nc.vector.memset(c_t[:, 0:8], 0.0)
            for t in range(0, nlev, TB):
                n = min(TB, nlev - t)
                child = (st[l + 1][0], st[l + 1][1], t)
                dst = (h_t, c_t, t // 2) if l > 0 else None
                _up_tile(nc, mybir, pools, L, W, xin, xout,
                         n, base + t, child, dst, base + t)
            if l > 0:
                st[l] = (h_t, c_t)


def _emit_dn(nc, tc, mybir, L, W, xin, xt0, xout, rsin, rsout, core_ids):
    F16 = mybir.dt.float16
    OP = mybir.AluOpType
    with (
        tc.tile_pool(name=f"dnx{L}", bufs=3) as xpool,
        tc.tile_pool(name=f"dng{L}", bufs=2, space="PSUM") as gpool,
        tc.tile_pool(name=f"dne{L}", bufs=2) as epool,
        tc.tile_pool(name=f"dnst{L}", bufs=1) as stpool,
    ):
        pools = (xpool, gpool, epool)
        st = None
        for l in range(0, SH_LV):                        # replicated top
            nlev = 2 ** l
            base = 2 ** l - 1
            w = 8192 if l % 2 == 0 else 4096
            sh_new = stpool.tile([64, w], F16, tag=f"dh{l % 2}")
            sc_new = stpool.tile([128, w], F16, tag=f"dc{l % 2}")
            if nlev < 8:
                nc.vector.memset(sh_new[:, 0:8], 0.0)
                nc.vector.memset(sc_new[64:128, 0:8], 0.0)
            for t in range(0, nlev, TB):
                n = min(TB, nlev - t)
                parent = None if l == 0 else (st[0], st[1], t // 2)
                _dn_tile(nc, mybir, pools, L, W, xin, xout,
                         n, base + t, parent, (sh_new, sc_new, t), base + t)
            st = (sh_new, sc_new)

        nc.sync.dma_start(
            rsin.ap().rearrange("(k r) j -> r k j", r=128)[0:64],
            st[0][:, 0:1024].rearrange("p (k j) -> p k j", k=8))
        nc.sync.dma_start(
            rsin.ap().rearrange("(k r) j -> r k j", r=128)[64:128],
            st[1][64:128, 0:1024].rearrange("p (k j) -> p k j", k=8))
        nc.gpsimd.collective_compute(
            "ReduceScatter", OP.max, replica_groups=[core_ids],
            ins=[rsin[:]], outs=[rsout[:]])
        psh = xpool.tile([64, 128], F16, tag="psh")
        nc.sync.dma_start(psh[:], rsout[0:64, :])
        psc = xpool.tile([128, 128], F16, tag="psc")
        nc.sync.dma_start(psc[64:128, :], rsout[64:128, :])
        st = (psh, psc)

        for l in range(SH_LV, D):                        # deep, sharded
            m = 2 ** (l - 3)
            last = l == D - 1
            if not last:
                w = 8192 if l % 2 == 0 else 4096
                sh_new = stpool.tile([64, w], F16, tag=f"dh{l % 2}")
                sc_new = stpool.tile([128, w], F16, tag=f"dc{l % 2}")
            for t in range(0, m, TB):
                n = min(TB, m - t)
                parent = (st[0], st[1], t // 2)
                dst = None if last else (sh_new, sc_new, t)
                _dn_tile(nc, mybir, pools, L, W, xin, xout,
                         n, _deep_off(l) + t, parent, dst, _deep_off(l) + t)
            if not last:
                st = (sh_new, sc_new)


# ------------------------------------------------------------------- driver
def _in_maps(features, params):
    feats_np = np.asarray(features, np.float32)
    fcores = _slice_features(feats_np)
    base = {
        "ident": np.eye(128, dtype=np.float32),
        "uscale": np.concatenate([np.ones((64, 1), np.float32),
                                  np.full((64, 1), 0.5, np.float32)]),
    }
    for L, lp in enumerate(params):
        wx, wlr, b = _prep_up(lp['fwd'])
        base[f"u{L}_wx"] = wx
        base[f"u{L}_wlr"] = wlr
        base[f"u{L}_b"] = b
        wx, wh, b = _prep_dn(lp['bwd'])
        base[f"d{L}_wx"] = wx
        base[f"d{L}_wh"] = wh
        base[f"d{L}_b"] = b
    base[f"bt_{_SRC_HASH}"] = np.zeros((1, 1), np.float32)
    return [dict(base, feats=fcores[k]) for k in range(NCORES)]


def kernel(features, params):
    from concourse.bass_utils import run_bass_kernel_spmd
    if "nc" not in _cache:
        _cache["nc"] = _build()
    nc = _cache["nc"]
    in_maps = _in_maps(features, params)
    res = run_bass_kernel_spmd(nc, in_maps, list(range(NCORES)))
    return _assemble([res.results[k]["outT"] for k in range(NCORES)])
